# revision 5
# baseline (speedup 1.0000x reference)
"""Grouped positional GEMM for Trainium2: y[b,s,:] = x[b,s,:] @ W[s] + bias[s].

Shards the position axis S=256 across 8 NeuronCores (32 positions/core).
Per position: y_s = x_s @ W_s + b_s with x_s [16,1024], W_s [1024,1024].
PE layout: out[m,n] += lhsT[k,m] * rhs[k,n] with lhsT = x_s^T chunk [128,16]
(pre-transposed on host), rhs = W_s chunk [128,512], accumulated over 8
k-chunks into PSUM [16,512]; bias added during the PSUM->SBUF copy on DVE.
"""

import numpy as np
import concourse.bass as bass  # noqa: F401  (bass must import before bacc)
from concourse import bacc
import concourse.mybir as mybir
from concourse.tile import TileContext
from concourse.bass_utils import run_bass_kernel_spmd

B, S, DIN, DOUT = 16, 256, 1024, 1024
NCORES = 8
SL = S // NCORES   # 32 positions per core
P = 128
KC = DIN // P      # 8 contraction chunks
NF = 512           # fp32 moving-operand max free size = one PSUM bank
NCH = DOUT // NF   # 2 output chunks

_cache = {}


def build(sl=SL, wbufs=3):
    nc = bacc.Bacc(None, target_bir_lowering=False)
    xT = nc.dram_tensor("xt", [P, sl, KC, B], mybir.dt.float32, kind="ExternalInput")
    W = nc.dram_tensor("w", [sl, DIN, DOUT], mybir.dt.float32, kind="ExternalInput")
    bb = nc.dram_tensor("bb", [sl, B, DOUT], mybir.dt.float32, kind="ExternalInput")
    y = nc.dram_tensor("y", [sl, B, DOUT], mybir.dt.float32, kind="ExternalOutput")

    with TileContext(nc) as tc:
        with (
            tc.tile_pool(name="xpool", bufs=1) as xpool,
            tc.tile_pool(name="bpool", bufs=2) as bpool,
            tc.tile_pool(name="wpool", bufs=wbufs) as wpool,
            tc.tile_pool(name="pp", bufs=8, space="PSUM") as pp,
            tc.tile_pool(name="opool", bufs=4) as opool,
        ):
            xt = xpool.tile([P, sl * KC * B], mybir.dt.float32)
            nc.sync.dma_start(out=xt[:], in_=xT.rearrange("p s k m -> p (s k m)"))
            for s in range(sl):
                wt = wpool.tile([P, KC * DOUT], mybir.dt.float32)
                nc.sync.dma_start(
                    out=wt[:].rearrange("p (k n) -> p k n", k=KC),
                    in_=W[s].rearrange("(k p) n -> p k n", p=P),
                )
                bt = bpool.tile([B, DOUT], mybir.dt.float32)
                nc.sync.dma_start(out=bt[:], in_=bb[s])
                for j in range(NCH):
                    ps = pp.tile([B, NF], mybir.dt.float32)
                    for k in range(KC):
                        nc.tensor.matmul(
                            ps[:],
                            lhsT=xt[:, (s * KC + k) * B : (s * KC + k + 1) * B],
                            rhs=wt[:, k * DOUT + j * NF : k * DOUT + j * NF + NF],
                            start=(k == 0),
                            stop=(k == KC - 1),
                        )
                    ot = opool.tile([B, NF], mybir.dt.float32)
                    nc.vector.tensor_add(ot[:], ps[:], bt[:, j * NF : (j + 1) * NF])
                    nc.sync.dma_start(out=y[s, :, j * NF : (j + 1) * NF], in_=ot[:])
    nc.compile()
    return nc


def make_in_maps(x, W, b):
    """Shard full inputs into per-core input maps (host-side prep)."""
    in_maps = []
    for c in range(NCORES):
        sel = slice(c * SL, (c + 1) * SL)
        # xt[p, s, k, m] = x[m, c*SL+s, k*P+p]
        xt = np.ascontiguousarray(
            x[:, sel, :].reshape(B, SL, KC, P).transpose(3, 1, 2, 0)
        )
        brep = np.ascontiguousarray(
            np.broadcast_to(b[sel][:, None, :], (SL, B, DOUT))
        )
        in_maps.append({"xt": xt, "w": W[sel], "bb": brep})
    return in_maps


def assemble(results):
    """Per-core y [SL, B, DOUT] -> full [B, S, DOUT]."""
    ys = np.concatenate([r["y"] for r in results], axis=0)  # [S, B, DOUT]
    return np.ascontiguousarray(ys.transpose(1, 0, 2))


def kernel(x, W, b):
    x = np.asarray(x, dtype=np.float32)
    W = np.asarray(W, dtype=np.float32)
    b = np.asarray(b, dtype=np.float32)
    if "nc" not in _cache:
        _cache["nc"] = build()
    nc = _cache["nc"]
    in_maps = make_in_maps(x, W, b)
    res = run_bass_kernel_spmd(nc, in_maps, core_ids=list(range(NCORES)))
    return assemble(res.results)


# revision 16
# speedup vs baseline: 156.9295x; 156.9295x over previous
"""Grouped positional GEMM for Trainium2: y[b,s,:] = x[b,s,:] @ W[s] + bias[s].

Shards the position axis S=256 across 8 NeuronCores (32 positions/core).
Per position: y_s = x_s @ W_s + b_s with x_s [16,1024], W_s [1024,1024].
PE layout: out[m,n] += lhsT[k,m] * rhs[k,n] with lhsT = x_s^T chunk [128,16]
(pre-transposed on host), rhs = W_s chunk [128,512], accumulated over 8
k-chunks into PSUM [16,512]; bias added during the PSUM->SBUF copy on DVE.
"""

import numpy as np
import concourse.bass as bass  # noqa: F401  (bass must import before bacc)
from concourse import bacc
import concourse.mybir as mybir
from concourse.tile import TileContext
from concourse.bass_utils import run_bass_kernel_spmd

B, S, DIN, DOUT = 16, 256, 1024, 1024
NCORES = 8
SL = S // NCORES   # 32 positions per core
P = 128
KC = DIN // P      # 8 contraction chunks
NF = 512           # fp32 moving-operand max free size = one PSUM bank
NCH = DOUT // NF   # 2 output chunks

_cache = {}


def build_f32r3(sl=SL, repeat=1, wbufs=2):
    """3-term float32r decomposition: y = xr@Wr + xr@We + xe@Wr.

    f32r (TF32-like, ~13-bit mantissa) matmuls run 4x faster than fp32 on the
    PE (1 cyc/row vs 4). Splitting both operands into rounded + residual keeps
    every retained product exact, so accuracy matches plain fp32 while PE time
    drops from ~437us to ~327us/core, putting the kernel at the HBM roofline.
    W rounds on ACT (copy), residual on DVE (sub), per half-position chunk.
    """
    f32 = mybir.dt.float32
    f32r = mybir.dt.float32r
    WH = KC // 2  # k-chunks per half-position chunk
    nc = bacc.Bacc(None, target_bir_lowering=False)
    xT = nc.dram_tensor("xt", [P, sl, KC, B], f32, kind="ExternalInput")
    W = nc.dram_tensor("w", [sl, DIN, DOUT], f32, kind="ExternalInput")
    bb = nc.dram_tensor("bb", [sl, B, DOUT], f32, kind="ExternalInput")
    y = nc.dram_tensor("y", [sl, B, DOUT], f32, kind="ExternalOutput")

    with TileContext(nc) as tc:
        from contextlib import ExitStack

        with (
            tc.tile_pool(name="xpool", bufs=1) as xpool,
            tc.tile_pool(name="xrpool", bufs=1) as xrpool,
            tc.tile_pool(name="xepool", bufs=1) as xepool,
            tc.tile_pool(name="bpool", bufs=2) as bpool,
            tc.tile_pool(name="w32pool", bufs=wbufs) as w32pool,
            tc.tile_pool(name="wrpool", bufs=wbufs) as wrpool,
            tc.tile_pool(name="wepool", bufs=wbufs) as wepool,
            tc.tile_pool(name="pp", bufs=8, space="PSUM") as pp,
            tc.tile_pool(name="opool", bufs=4) as opool,
            ExitStack() as es,
        ):
            if repeat > 1:
                es.enter_context(tc.For_i(0, repeat, 1))
            xt = xpool.tile([P, sl * KC * B], f32)
            nc.sync.dma_start(out=xt[:], in_=xT.rearrange("p s k m -> p (s k m)"))
            xtr = xrpool.tile([P, sl * KC * B], f32r)
            nc.scalar.copy(xtr[:], xt[:])
            xte = xepool.tile([P, sl * KC * B], f32r)
            nc.vector.tensor_sub(xte[:], xt[:], xtr[:].bitcast(f32))

            def xr_s(kg, s):
                return xtr[:, (s * KC + kg) * B : (s * KC + kg + 1) * B]

            def xe_s(kg, s):
                return xte[:, (s * KC + kg) * B : (s * KC + kg + 1) * B]

            for s in range(sl):
                ps = [pp.tile([B, NF], f32, name=f"ps{j}_{s}", tag="ps") for j in range(NCH)]
                for h in range(2):
                    w32 = w32pool.tile([P, WH * DOUT], f32)
                    nc.sync.dma_start(
                        out=w32[:].rearrange("p (k n) -> p k n", k=WH),
                        in_=W[s, h * WH * P : (h + 1) * WH * P, :].rearrange(
                            "(k p) n -> p k n", p=P
                        ),
                    )
                    wtr = wrpool.tile([P, WH * DOUT], f32r)
                    nc.scalar.copy(wtr[:], w32[:])
                    wte = wepool.tile([P, WH * DOUT], f32r)
                    nc.vector.tensor_sub(wte[:], w32[:], wtr[:].bitcast(f32))
                    for k in range(WH):
                        kg = h * WH + k
                        for j in range(NCH):
                            first = h == 0 and k == 0
                            last = h == 1 and k == WH - 1
                            rr = wtr[:, k * DOUT + j * NF : k * DOUT + j * NF + NF]
                            re = wte[:, k * DOUT + j * NF : k * DOUT + j * NF + NF]
                            nc.tensor.matmul(
                                ps[j][:], lhsT=xr_s(kg, s), rhs=rr,
                                start=first, stop=False, skip_group_check=True,
                            )
                            nc.tensor.matmul(
                                ps[j][:], lhsT=xr_s(kg, s), rhs=re,
                                start=False, stop=False, skip_group_check=True,
                            )
                            nc.tensor.matmul(
                                ps[j][:], lhsT=xe_s(kg, s), rhs=rr,
                                start=False, stop=last, skip_group_check=True,
                            )
                bt = bpool.tile([B, DOUT], f32)
                nc.sync.dma_start(out=bt[:], in_=bb[s])
                for j in range(NCH):
                    ot = opool.tile([B, NF], f32)
                    nc.vector.tensor_add(ot[:], ps[j][:], bt[:, j * NF : (j + 1) * NF])
                    nc.sync.dma_start(out=y[s, :, j * NF : (j + 1) * NF], in_=ot[:])
    nc.compile()
    return nc


SCALE = 1024.0  # input scale: keeps fp16 residuals out of the denormal range
DESCALE = 1.0 / (SCALE * SCALE)


def build_fp16_3(sl=SL, repeat=1, wbufs=3, spread_dma=True):
    """3-term fp16 decomposition, split on host: y = xh@Wh + xh@Wl + xl@Wh.

    Host sends Wh=fp16(W*SC), Wl=fp16(W*SC - Wh) (same total bytes as fp32 W)
    plus xh/xl likewise. Every retained product is near-exact; the dropped
    xl@Wl term is ~2^-22 relative, so accuracy matches plain fp32 while PE
    time is 3 cyc/row instead of fp32's 4*2 (24 x 512-row 1-cyc matmuls per
    position vs 16 x 512-row 4-cyc). The kernel does no datatype conversions
    on device; bias add + 1/SC^2 descale fuse into one DVE op per out tile.
    """
    f32 = mybir.dt.float32
    f16 = mybir.dt.float16
    nc = bacc.Bacc(None, target_bir_lowering=False)
    xh = nc.dram_tensor("xh", [P, sl, KC, B], f16, kind="ExternalInput")
    xl = nc.dram_tensor("xl", [P, sl, KC, B], f16, kind="ExternalInput")
    wh = nc.dram_tensor("wh", [sl, DIN, DOUT], f16, kind="ExternalInput")
    wl = nc.dram_tensor("wl", [sl, DIN, DOUT], f16, kind="ExternalInput")
    bb = nc.dram_tensor("bb", [sl, B, DOUT], f32, kind="ExternalInput")
    y = nc.dram_tensor("y", [sl, B, DOUT], f32, kind="ExternalOutput")

    with TileContext(nc) as tc:
        from contextlib import ExitStack

        with (
            tc.tile_pool(name="xpool", bufs=1) as xpool,
            tc.tile_pool(name="bpool", bufs=2) as bpool,
            tc.tile_pool(name="whpool", bufs=wbufs) as whpool,
            tc.tile_pool(name="wlpool", bufs=wbufs) as wlpool,
            tc.tile_pool(name="pp", bufs=8, space="PSUM") as pp,
            tc.tile_pool(name="opool", bufs=4) as opool,
            ExitStack() as es,
        ):
            if repeat > 1:
                es.enter_context(tc.For_i(0, repeat, 1))
            xht = xpool.tile([P, sl * KC * B], f16, name="xht")
            nc.sync.dma_start(out=xht[:], in_=xh.rearrange("p s k m -> p (s k m)"))
            xlt = xpool.tile([P, sl * KC * B], f16, name="xlt")
            nc.sync.dma_start(out=xlt[:], in_=xl.rearrange("p s k m -> p (s k m)"))

            def xs(t, kg, s):
                return t[:, (s * KC + kg) * B : (s * KC + kg + 1) * B]

            weng = nc.scalar if spread_dma else nc.sync
            beng = nc.gpsimd if spread_dma else nc.sync
            for s in range(sl):
                wht = whpool.tile([P, KC * DOUT], f16)
                nc.sync.dma_start(
                    out=wht[:].rearrange("p (k n) -> p k n", k=KC),
                    in_=wh[s].rearrange("(k p) n -> p k n", p=P),
                )
                wlt = wlpool.tile([P, KC * DOUT], f16)
                weng.dma_start(
                    out=wlt[:].rearrange("p (k n) -> p k n", k=KC),
                    in_=wl[s].rearrange("(k p) n -> p k n", p=P),
                )
                bt = bpool.tile([B, DOUT], f32)
                beng.dma_start(out=bt[:], in_=bb[s])
                for j in range(NCH):
                    ps = pp.tile([B, NF], f32, tag="ps")
                    for k in range(KC):
                        rh = wht[:, k * DOUT + j * NF : k * DOUT + j * NF + NF]
                        rl = wlt[:, k * DOUT + j * NF : k * DOUT + j * NF + NF]
                        nc.tensor.matmul(
                            ps[:], lhsT=xs(xht, k, s), rhs=rh,
                            start=(k == 0), stop=False, skip_group_check=True,
                        )
                        nc.tensor.matmul(
                            ps[:], lhsT=xs(xht, k, s), rhs=rl,
                            start=False, stop=False, skip_group_check=True,
                        )
                        nc.tensor.matmul(
                            ps[:], lhsT=xs(xlt, k, s), rhs=rh,
                            start=False, stop=(k == KC - 1), skip_group_check=True,
                        )
                    ot = opool.tile([B, NF], f32)
                    nc.vector.scalar_tensor_tensor(
                        ot[:], ps[:], DESCALE, bt[:, j * NF : (j + 1) * NF],
                        op0=mybir.AluOpType.mult, op1=mybir.AluOpType.add,
                    )
                    oeng = (nc.sync, weng)[j % 2] if spread_dma else nc.sync
                    oeng.dma_start(out=y[s, :, j * NF : (j + 1) * NF], in_=ot[:])
    nc.compile()
    return nc


def build(sl=SL, wbufs=3, repeat=1, wsplit=1, bias_engine="sync"):
    nc = bacc.Bacc(None, target_bir_lowering=False)
    xT = nc.dram_tensor("xt", [P, sl, KC, B], mybir.dt.float32, kind="ExternalInput")
    W = nc.dram_tensor("w", [sl, DIN, DOUT], mybir.dt.float32, kind="ExternalInput")
    bb = nc.dram_tensor("bb", [sl, B, DOUT], mybir.dt.float32, kind="ExternalInput")
    y = nc.dram_tensor("y", [sl, B, DOUT], mybir.dt.float32, kind="ExternalOutput")

    with TileContext(nc) as tc:
        from contextlib import ExitStack

        with (
            tc.tile_pool(name="xpool", bufs=1) as xpool,
            tc.tile_pool(name="bpool", bufs=2) as bpool,
            tc.tile_pool(name="wpool", bufs=wbufs) as wpool,
            tc.tile_pool(name="pp", bufs=8, space="PSUM") as pp,
            tc.tile_pool(name="opool", bufs=4) as opool,
            ExitStack() as es,
        ):
            if repeat > 1:
                es.enter_context(tc.For_i(0, repeat, 1))
            xt = xpool.tile([P, sl * KC * B], mybir.dt.float32)
            nc.sync.dma_start(out=xt[:], in_=xT.rearrange("p s k m -> p (s k m)"))
            for s in range(sl):
                wt = wpool.tile([P, KC * DOUT], mybir.dt.float32)
                kstep = KC // wsplit
                for w_i in range(wsplit):
                    k0 = w_i * kstep
                    nc.sync.dma_start(
                        out=wt[:, k0 * DOUT : (k0 + kstep) * DOUT].rearrange(
                            "p (k n) -> p k n", k=kstep
                        ),
                        in_=W[s, k0 * P : (k0 + kstep) * P, :].rearrange(
                            "(k p) n -> p k n", p=P
                        ),
                    )
                bt = bpool.tile([B, DOUT], mybir.dt.float32)
                getattr(nc, bias_engine).dma_start(out=bt[:], in_=bb[s])
                for j in range(NCH):
                    ps = pp.tile([B, NF], mybir.dt.float32)
                    for k in range(KC):
                        nc.tensor.matmul(
                            ps[:],
                            lhsT=xt[:, (s * KC + k) * B : (s * KC + k + 1) * B],
                            rhs=wt[:, k * DOUT + j * NF : k * DOUT + j * NF + NF],
                            start=(k == 0),
                            stop=(k == KC - 1),
                        )
                    ot = opool.tile([B, NF], mybir.dt.float32)
                    nc.vector.tensor_add(ot[:], ps[:], bt[:, j * NF : (j + 1) * NF])
                    nc.sync.dma_start(out=y[s, :, j * NF : (j + 1) * NF], in_=ot[:])
    nc.compile()
    return nc


def _xpose(a, sl):
    """[B, sl, DIN] -> [P, sl, KC, B] (partition-major for a contiguous DMA)."""
    return np.ascontiguousarray(a.reshape(B, sl, KC, P).transpose(3, 1, 2, 0))


def make_in_maps(x, W, b, mode="fp32"):
    """Shard full inputs into per-core input maps (host-side prep)."""
    in_maps = []
    if mode == "fp16_3":
        xs = x * SCALE
        xh = xs.astype(np.float16)
        xl = (xs - xh.astype(np.float32)).astype(np.float16)
        Ws = W * SCALE
        Wh = Ws.astype(np.float16)
        Wl = (Ws - Wh.astype(np.float32)).astype(np.float16)
    for c in range(NCORES):
        sel = slice(c * SL, (c + 1) * SL)
        brep = np.ascontiguousarray(
            np.broadcast_to(b[sel][:, None, :], (SL, B, DOUT))
        )
        if mode == "fp16_3":
            in_maps.append(
                {
                    "xh": _xpose(xh[:, sel, :], SL),
                    "xl": _xpose(xl[:, sel, :], SL),
                    "wh": Wh[sel],
                    "wl": Wl[sel],
                    "bb": brep,
                }
            )
        else:
            in_maps.append({"xt": _xpose(x[:, sel, :], SL), "w": W[sel], "bb": brep})
    return in_maps


def assemble(results):
    """Per-core y [SL, B, DOUT] -> full [B, S, DOUT]."""
    ys = np.concatenate([r["y"] for r in results], axis=0)  # [S, B, DOUT]
    return np.ascontiguousarray(ys.transpose(1, 0, 2))


MODE = "fp16_3"


def BUILDER(**kw):
    return build_fp16_3(**kw)


def kernel(x, W, b):
    x = np.asarray(x, dtype=np.float32)
    W = np.asarray(W, dtype=np.float32)
    b = np.asarray(b, dtype=np.float32)
    if "nc" not in _cache:
        _cache["nc"] = BUILDER(sl=SL)
    nc = _cache["nc"]
    in_maps = make_in_maps(x, W, b, mode=MODE)
    res = run_bass_kernel_spmd(nc, in_maps, core_ids=list(range(NCORES)))
    return assemble(res.results)
